# revision 44
# baseline (speedup 1.0000x reference)
"""Doc-masked causal multi-head attention on TRN2, 8-core SPMD.

Sharding: core c -> batch b = c//4, heads [4*(c%4), 4*(c%4)+4).
Single pass over the core's 4 heads.  Host pre-converts x/W_qkv/W_out and
the RoPE tables to bf16 (and pre-transposes the tables), so weights and
activations DMA straight into SBUF with no on-device staging/transposes.
Each 512-row query group tt runs: x^T PE-transpose -> q/k projections with
fused RoPE -> v projection -> doc-causal attention (transposed S^T layout,
ones-matmul denominators) -> output projection against this core's 512
rows of W_out, DMAed out per 128-row block.  The host sums the 4 partials
per batch.

Doc masks are built once per (group, jblk) tile and shared by the 4 heads:
a tensor_scalar (i < doc_end) compare on DVE plus, for diagonal tiles, a
causal affine_select on GpSimd; each head then applies one bf16 mul.
Block-sparsity: (group, jblk) tiles outside every document's causal band
are skipped at trace time based on the actual doc_ids.

Matmuls run in bf16 (fp32 accumulation in PSUM); softmax math in fp32.
"""

import os
import sys

import numpy as np

for _p in ("/opt/trn_rl_repo", "/root/.axon_site/_ro/trn_rl_repo"):
    if os.path.isdir(_p) and _p not in sys.path:
        sys.path.append(_p)

import concourse.bass as bass
from concourse import bacc
import concourse.tile as tile
from concourse import mybir
from concourse.bass_utils import run_bass_kernel_spmd

B, T, D, H, HD = 2, 2048, 2048, 16, 128
NCORES = 8
NH = 4  # heads per core
TT = T // 512  # 4 query groups of 512 rows
KB = D // 128  # 16 contraction blocks
NJB = T // 128  # 16 j-blocks
SCALE = 1.0 / float(np.sqrt(HD))

F32 = mybir.dt.float32
F32R = mybir.dt.float32r
BF16 = mybir.dt.bfloat16
I32 = mybir.dt.int32
AF = mybir.ActivationFunctionType
ALU = mybir.AluOpType
NPBF16 = mybir.dt.np(BF16)


def _doc_ends(doc_row: np.ndarray) -> np.ndarray:
    """e[i] = one past the last index of the document containing row i."""
    e = np.zeros(T, np.int64)
    end = T
    for i in range(T - 1, -1, -1):
        if i < T - 1 and doc_row[i] != doc_row[i + 1]:
            end = i + 1
        e[i] = end
    return e


def _tile_structure(e_by_batch):
    """(group, jblk) -> (kind, c0, w_hi); tiles skippable for both batches
    are omitted.  group = 512 query rows, jblk = 128 key rows.  Columns
    outside [c0, w_hi) are fully masked for every batch: c0 from causality,
    w_hi from the last document end in the block."""
    struct = {}
    for g in range(TT):
        i_lo, i_hi = g * 512, g * 512 + 511
        tiles = {}
        for jblk in range(0, (g + 1) * 4):
            j_lo, j_hi = jblk * 128, jblk * 128 + 127
            valid = any(
                j_hi >= i_lo or int(e[j_hi]) > i_lo for e in e_by_batch
            )
            if not valid:
                continue
            full = all(
                j_hi <= i_lo and i_hi < int(e[j_lo]) for e in e_by_batch
            )
            c0 = max(0, j_lo - i_lo)
            me = max(
                int(max(e[j_lo : j_hi + 1])) for e in e_by_batch
            )
            w_hi = min(512, max(c0, me - i_lo))
            tiles[jblk] = ("full" if full else "bound", c0, w_hi)
        struct[g] = tiles
    return struct


def build_program(doc_ids: np.ndarray, repeat: int = 1):
    e_by_batch = [_doc_ends(np.asarray(doc_ids[b])) for b in range(B)]
    struct = _tile_structure(e_by_batch)

    nc = bacc.Bacc("TRN2", debug=False)
    x_d = nc.dram_tensor("x_in", [T, D], BF16, kind="ExternalInput").ap()
    w_d = nc.dram_tensor("w_in", [D, 12 * 128], BF16, kind="ExternalInput").ap()
    wo_d = nc.dram_tensor("wout_in", [NH * HD, D], BF16, kind="ExternalInput").ap()
    cos_d = nc.dram_tensor("cosT_in", [128, T], BF16, kind="ExternalInput").ap()
    sin_d = nc.dram_tensor("sinS_in", [128, T], BF16, kind="ExternalInput").ap()
    e_d = nc.dram_tensor("e_in", [128, NJB * TT], F32, kind="ExternalInput").ap()
    out_d = nc.dram_tensor("out_p", [T, D], BF16, kind="ExternalOutput").ap()

    _cp = [0]

    def copy_any(out, in_):
        # PSUM-evacuation copies: only Act/DVE may read PSUM
        _cp[0] ^= 1
        if _cp[0]:
            nc.scalar.copy(out, in_)
        else:
            nc.vector.tensor_copy(out, in_)

    with tile.TileContext(nc) as tc:
        from contextlib import ExitStack

        with ExitStack() as ctx:
            consts = ctx.enter_context(tc.tile_pool(name="consts", bufs=1))
            pp = ctx.enter_context(tc.tile_pool(name="pp", bufs=1, space="PSUM"))
            wpool = ctx.enter_context(tc.tile_pool(name="wpool", bufs=1))
            kvpool = ctx.enter_context(tc.tile_pool(name="kvpool", bufs=1))
            xtp = ctx.enter_context(tc.tile_pool(name="xt", bufs=2))
            qt_pool = ctx.enter_context(tc.tile_pool(name="qt_pool", bufs=2))
            ot_pool = ctx.enter_context(tc.tile_pool(name="ot_pool", bufs=2))
            rope_pool = ctx.enter_context(tc.tile_pool(name="rope", bufs=2))
            pt_pool = ctx.enter_context(tc.tile_pool(name="pt_pool", bufs=6))
            mask_pool = ctx.enter_context(tc.tile_pool(name="mask", bufs=7))
            small = ctx.enter_context(tc.tile_pool(name="small", bufs=2))
            fin_pool = ctx.enter_context(tc.tile_pool(name="fin", bufs=3))

            # ---- constants (one-time) ----
            ones_bf = consts.tile([128, 1], BF16)
            nc.vector.memset(ones_bf, 1.0)
            cosT = consts.tile([128, T], BF16)
            sinS = consts.tile([128, T], BF16)
            e_sb = consts.tile([128, NJB * TT], F32)
            nc.sync.dma_start(e_sb, e_d)
            # integer iota compared against integer doc-ends directly
            iota_l = consts.tile([128, 512], I32)
            nc.gpsimd.iota(
                iota_l, pattern=[[1, 512]], base=0, channel_multiplier=0
            )

            pending = None  # deferred output projection (outt, tt, wout)
            for _rep in range(repeat):
                # x rows for the first group go out first so the PE can
                # start transposing ASAP; the q/k weight columns stream
                # per-kb behind them (the first projection chunk tracks the
                # stream), then v columns, then W_out (needed much later).
                xt0 = xtp.tile([128, KB, 512], BF16, tag="xt")
                nc.sync.dma_start(
                    xt0[:, 0 : KB // 2, :], x_d[0:512, 0 : D // 2],
                    transpose=True,
                )
                nc.sync.dma_start(
                    xt0[:, KB // 2 :, :], x_d[0:512, D // 2 :],
                    transpose=True,
                )
                if _rep == 0:
                    # RoPE tables land after the first x rows but before the
                    # q/k weights finish streaming
                    nc.sync.dma_start(cosT, cos_d)
                    nc.sync.dma_start(sinS, sin_d)
                if pending is not None:
                    # carried-over output projection of the previous rep's
                    # last group: runs while this rep's weights stream in,
                    # and its output DMAs beat them into the queue
                    emit_outproj(*pending)
                    pending = None
                w_bf = wpool.tile([128, KB, 12 * 128], BF16, tag="w")
                nc.sync.dma_start(
                    w_bf[:, :, 0 : 8 * 128],
                    w_d[:, 0 : 8 * 128].rearrange("(kb p) c -> p kb c", p=128),
                )
                nc.sync.dma_start(
                    w_bf[:, :, 8 * 128 : 12 * 128],
                    w_d[:, 8 * 128 : 12 * 128].rearrange(
                        "(kb p) c -> p kb c", p=128
                    ),
                )
                wout_bf = wpool.tile([128, NH, D], BF16, tag="wo", bufs=2)
                nc.sync.dma_start(
                    wout_bf, wo_d.rearrange("(a p) d -> p a d", p=128)
                )
                kT = kvpool.tile([128, NH, T], BF16, tag="kT")
                v_sb = kvpool.tile([128, NJB, NH * 128], BF16, tag="v_sb")

                def emit_outproj(outt, tt, wout_bf):
                    for lt in range(4):
                        tg = tt * 4 + lt
                        fin = fin_pool.tile([128, D], BF16, tag="fin")
                        for nt in range(4):
                            fp = pp.tile([128, 512], F32, tag="mm", bufs=2)
                            for h in range(NH):
                                nc.tensor.matmul(
                                    fp,
                                    lhsT=outt[:, h, lt * 128 : (lt + 1) * 128],
                                    rhs=wout_bf[:, h, nt * 512 : (nt + 1) * 512],
                                    start=(h == 0),
                                    stop=(h == NH - 1),
                                )
                            copy_any(fin[:, nt * 512 : (nt + 1) * 512], fp)
                        nc.sync.dma_start(
                            out_d[tg * 128 : (tg + 1) * 128, :], fin
                        )

                for tt in range(TT):
                    t0 = tt * 512
                    # -- x^T tile via XBAR DMA transpose:
                    # xt[:, kb, :] = x[t0:t0+512, kb-block].T --
                    if tt == 0:
                        xt_slab = xt0
                    else:
                        xt_slab = xtp.tile([128, KB, 512], BF16, tag="xt")
                        nc.sync.dma_start(
                            xt_slab[:, 0 : KB // 2, :],
                            x_d[t0 : t0 + 512, 0 : D // 2],
                            transpose=True,
                        )
                        nc.sync.dma_start(
                            xt_slab[:, KB // 2 :, :],
                            x_d[t0 : t0 + 512, D // 2 :],
                            transpose=True,
                        )

                    # -- doc-causal masks, shared across the 4 heads; built
                    # here so DVE/GpSimd do them while the PE projects --
                    tiles = struct[tt]
                    jblks = sorted(tiles)
                    widest_jb = max(
                        jblks, key=lambda j: tiles[j][2] - tiles[j][1]
                    )
                    masks = {}
                    for jblk in jblks:
                        if tiles[jblk][0] != "bound":
                            continue
                        if jblk == widest_jb:
                            mc0, mw_hi = 0, 512
                        else:
                            _, mc0, mw_hi = tiles[jblk]
                        mw = mw_hi - mc0
                        m = mask_pool.tile([128, 512], BF16, tag="m")
                        # keep iff i < doc_end(j):  l < e[j] - tt*512
                        nc.vector.tensor_scalar(
                            m[:, mc0:mw_hi],
                            iota_l[:, mc0:mw_hi],
                            e_sb[:, jblk * TT + tt : jblk * TT + tt + 1],
                            None,
                            ALU.is_lt,
                        )
                        if jblk >= 4 * tt:
                            # causal half: keep iff i - j >= 0
                            nc.gpsimd.affine_select(
                                out=m[:, mc0:mw_hi],
                                in_=m[:, mc0:mw_hi],
                                compare_op=ALU.is_ge,
                                fill=0.0,
                                base=tt * 512 + mc0 - jblk * 128,
                                channel_multiplier=-1,
                                pattern=[[1, mw]],
                            )
                        masks[jblk] = m

                    # -- q/k projections + RoPE (transposed layout) --
                    qT = qt_pool.tile([128, NH, 512], BF16, tag="qT")
                    for hl in range(NH):
                        for qk in range(2):  # 0 = q, 1 = k
                            chunk = qk * NH + hl
                            ps = pp.tile([128, 512], F32, tag="mm", bufs=2)
                            for kb in range(KB):
                                nc.tensor.matmul(
                                    ps,
                                    lhsT=w_bf[
                                        :, kb, chunk * 128 : (chunk + 1) * 128
                                    ],
                                    rhs=xt_slab[:, kb, :],
                                    start=(kb == 0),
                                    stop=(kb == KB - 1),
                                )
                            # RoPE fused on the PSUM result: cos-product on
                            # GpSimd, the rotate_half sin-products as two
                            # crossed-partition DVE muls (sign baked into the
                            # host-prepared sinS table), then one add.
                            raw = rope_pool.tile([128, 512], BF16, tag="raw")
                            nc.scalar.copy(raw, ps)
                            tmpc = rope_pool.tile([128, 512], BF16, tag="tmpc")
                            nc.gpsimd.tensor_mul(
                                tmpc, raw, cosT[:, t0 : t0 + 512]
                            )
                            sp = rope_pool.tile([128, 512], BF16, tag="sp")
                            nc.vector.tensor_mul(
                                sp[0:64, :], ps[64:128, :],
                                sinS[0:64, t0 : t0 + 512],
                            )
                            nc.vector.tensor_mul(
                                sp[64:128, :], ps[0:64, :],
                                sinS[64:128, t0 : t0 + 512],
                            )
                            dst = (
                                qT[:, hl, :]
                                if qk == 0
                                else kT[:, hl, t0 : t0 + 512]
                            )
                            nc.vector.tensor_add(dst, sp, tmpc)

                    # -- v projection (natural layout, 4 heads wide) --
                    for ts in range(4):
                        tb = tt * 4 + ts
                        ps = pp.tile([128, 512], F32, tag="mm", bufs=2)
                        for kb in range(KB):
                            nc.tensor.matmul(
                                ps,
                                lhsT=xt_slab[:, kb, ts * 128 : (ts + 1) * 128],
                                rhs=w_bf[:, kb, 8 * 128 : 12 * 128],
                                start=(kb == 0),
                                stop=(kb == KB - 1),
                            )
                        copy_any(v_sb[:, tb, :], ps)

                    # -- deferred output projection for the previous group:
                    # its matmuls cover the tail of that group's softmax
                    # normalize chain --
                    if pending is not None:
                        emit_outproj(*pending)
                        pending = None

                    # -- attention for this 512-row group, st two ahead.
                    # Tiles are width-trimmed to [c0, w_hi): columns below
                    # c0 are causally dead, columns past w_hi are past every
                    # document end.  The accumulation's first tile must run
                    # full width (start=True marks the whole PSUM zero
                    # region; its masked columns are zero anyway), so the
                    # widest tile goes first to minimize the trim loss. --
                    ordered = [widest_jb] + [
                        j for j in jblks if j != widest_jb
                    ]
                    pairs = [(hl, j) for hl in range(NH) for j in ordered]

                    def bounds_of(jblk, first):
                        if first:
                            return 0, 512
                        _, c0, w_hi = tiles[jblk]
                        return c0, w_hi

                    def emit_st(hl, jblk, first):
                        c0, w_hi = bounds_of(jblk, first)
                        st = pp.tile([128, 512], F32, tag="st", bufs=3)
                        nc.tensor.matmul(
                            st[:, 0 : w_hi - c0],
                            lhsT=kT[:, hl, jblk * 128 : (jblk + 1) * 128],
                            rhs=qT[:, hl, c0:w_hi],
                            start=True,
                            stop=True,
                        )
                        return st

                    nj = len(ordered)
                    outt = ot_pool.tile([128, NH, 512], BF16, tag="outt")
                    sts = [
                        emit_st(*pairs[0], True),
                        emit_st(*pairs[1], 1 % nj == 0),
                    ]
                    ones_ps = pv = None
                    for idx, (hl, jblk) in enumerate(pairs):
                        first = idx % nj == 0
                        last = idx % nj == nj - 1
                        if idx + 2 < len(pairs):
                            sts.append(
                                emit_st(*pairs[idx + 2], (idx + 2) % nj == 0)
                            )
                        st = sts[idx]
                        c0, w_hi = bounds_of(jblk, first)
                        w = w_hi - c0
                        slab = pt_pool.tile([128, 512], BF16, tag="pt")
                        nc.scalar.activation(
                            slab[:, 0:w], st[:, 0:w], AF.Exp, scale=SCALE
                        )
                        if jblk in masks:
                            nc.vector.tensor_mul(
                                slab[:, 0:w], slab[:, 0:w],
                                masks[jblk][:, c0:w_hi],
                            )
                        if first:
                            ones_ps = pp.tile([1, 512], F32, tag="ones", bufs=1)
                            pv = pp.tile([128, 512], F32, tag="pv", bufs=2)
                        nc.tensor.matmul(
                            ones_ps[:, c0:w_hi], lhsT=ones_bf, rhs=slab[:, 0:w],
                            start=first, stop=last,
                        )
                        nc.tensor.matmul(
                            pv[:, c0:w_hi],
                            lhsT=v_sb[:, jblk, hl * 128 : (hl + 1) * 128],
                            rhs=slab[:, 0:w],
                            start=first,
                            stop=last,
                        )
                        if last:
                            rc = small.tile([1, 512], F32, tag="rc", bufs=1)
                            nc.vector.reciprocal(rc, ones_ps)
                            rb = small.tile([128, 512], F32, tag="rb")
                            nc.gpsimd.partition_broadcast(rb, rc)
                            nc.vector.tensor_mul(outt[:, hl, :], pv, rb)

                    pending = (outt, tt, wout_bf)
            emit_outproj(*pending)
    nc.compile()
    return nc


def _core_in_map(c, x, sin, cos, W_qkv, W_out, doc_ids):
    b = c // 4
    h0 = (c % 4) * 4
    wq = W_qkv[:, h0 * 128 : (h0 + 4) * 128]
    wk = W_qkv[:, D + h0 * 128 : D + (h0 + 4) * 128]
    wv = W_qkv[:, 2 * D + h0 * 128 : 2 * D + (h0 + 4) * 128]
    w_in = np.concatenate([wq, wk, wv], axis=1).astype(NPBF16)

    sinS = np.asarray(sin, np.float32).T.copy()
    sinS[0:64] *= -1.0

    e = _doc_ends(np.asarray(doc_ids[b])).astype(np.float32)
    # e_g[p, jblk*TT + g] = e[jblk*128 + p] - g*512
    e_g = (
        e.reshape(NJB, 128).T[:, :, None]
        - (np.arange(TT, dtype=np.float32) * 512.0)[None, None, :]
    ).reshape(128, NJB * TT)

    return {
        "x_in": np.ascontiguousarray(x[b]).astype(NPBF16),
        "w_in": np.ascontiguousarray(w_in),
        "wout_in": np.ascontiguousarray(
            W_out[h0 * 128 : (h0 + 4) * 128, :]
        ).astype(NPBF16),
        "cosT_in": np.ascontiguousarray(np.asarray(cos, np.float32).T).astype(
            NPBF16
        ),
        "sinS_in": np.ascontiguousarray(sinS).astype(NPBF16),
        "e_in": np.ascontiguousarray(e_g, dtype=np.float32),
    }


_last_results = None


def kernel(x, sin, cos, W_qkv, W_out, doc_ids):
    x = np.asarray(x, np.float32)
    sin = np.asarray(sin, np.float32)
    cos = np.asarray(cos, np.float32)
    W_qkv = np.asarray(W_qkv, np.float32)
    W_out = np.asarray(W_out, np.float32)
    doc_ids = np.asarray(doc_ids)

    nc = build_program(doc_ids)
    in_maps = [
        _core_in_map(c, x, sin, cos, W_qkv, W_out, doc_ids) for c in range(NCORES)
    ]
    res = run_bass_kernel_spmd(nc, in_maps, core_ids=list(range(NCORES)))
    global _last_results
    _last_results = res
    outs = [np.asarray(res.results[c]["out_p"], np.float32) for c in range(NCORES)]
    out = np.stack(
        [
            outs[0] + outs[1] + outs[2] + outs[3],
            outs[4] + outs[5] + outs[6] + outs[7],
        ]
    )
    return out.astype(np.float32)


# revision 47
# speedup vs baseline: 2.2181x; 2.2181x over previous
"""Doc-masked causal multi-head attention on TRN2, 8-core SPMD.

Sharding: core c -> batch b = c//4, heads [4*(c%4), 4*(c%4)+4).
Single pass over the core's 4 heads.  Host pre-converts x/W_qkv/W_out and
the RoPE tables to bf16 (and pre-transposes the tables), so weights and
activations DMA straight into SBUF with no on-device staging/transposes.
Each 512-row query group tt runs: x^T PE-transpose -> q/k projections with
fused RoPE -> v projection -> doc-causal attention (transposed S^T layout,
ones-matmul denominators) -> output projection against this core's 512
rows of W_out, DMAed out per 128-row block.  The host sums the 4 partials
per batch.

Doc masks are built once per (group, jblk) tile and shared by the 4 heads:
a tensor_scalar (i < doc_end) compare on DVE plus, for diagonal tiles, a
causal affine_select on GpSimd; each head then applies one bf16 mul.
Block-sparsity: (group, jblk) tiles outside every document's causal band
are skipped at trace time based on the actual doc_ids.

Matmuls run in bf16 (fp32 accumulation in PSUM); softmax math in fp32.
"""

import os
import sys

import numpy as np

for _p in ("/opt/trn_rl_repo", "/root/.axon_site/_ro/trn_rl_repo"):
    if os.path.isdir(_p) and _p not in sys.path:
        sys.path.append(_p)

import concourse.bass as bass
from concourse import bacc
import concourse.tile as tile
from concourse import mybir
from concourse.bass_utils import run_bass_kernel_spmd

B, T, D, H, HD = 2, 2048, 2048, 16, 128
NCORES = 8
NH = 4  # heads per core
TT = T // 512  # 4 query groups of 512 rows
KB = D // 128  # 16 contraction blocks
NJB = T // 128  # 16 j-blocks
SCALE = 1.0 / float(np.sqrt(HD))

F32 = mybir.dt.float32
F32R = mybir.dt.float32r
BF16 = mybir.dt.bfloat16
I32 = mybir.dt.int32
AF = mybir.ActivationFunctionType
ALU = mybir.AluOpType
NPBF16 = mybir.dt.np(BF16)


def _doc_ends(doc_row: np.ndarray) -> np.ndarray:
    """e[i] = one past the last index of the document containing row i."""
    e = np.zeros(T, np.int64)
    end = T
    for i in range(T - 1, -1, -1):
        if i < T - 1 and doc_row[i] != doc_row[i + 1]:
            end = i + 1
        e[i] = end
    return e


def _tile_structure(e_by_batch):
    """(group, jblk) -> (kind, c0, w_hi); tiles skippable for both batches
    are omitted.  group = 512 query rows, jblk = 128 key rows.  Columns
    outside [c0, w_hi) are fully masked for every batch: c0 from causality,
    w_hi from the last document end in the block."""
    struct = {}
    for g in range(TT):
        i_lo, i_hi = g * 512, g * 512 + 511
        tiles = {}
        for jblk in range(0, (g + 1) * 4):
            j_lo, j_hi = jblk * 128, jblk * 128 + 127
            valid = any(
                j_hi >= i_lo or int(e[j_hi]) > i_lo for e in e_by_batch
            )
            if not valid:
                continue
            full = all(
                j_hi <= i_lo and i_hi < int(e[j_lo]) for e in e_by_batch
            )
            c0 = max(0, j_lo - i_lo)
            me = max(
                int(max(e[j_lo : j_hi + 1])) for e in e_by_batch
            )
            w_hi = min(512, max(c0, me - i_lo))
            tiles[jblk] = ("full" if full else "bound", c0, w_hi)
        struct[g] = tiles
    return struct


def build_program(doc_ids: np.ndarray, repeat: int = 1):
    e_by_batch = [_doc_ends(np.asarray(doc_ids[b])) for b in range(B)]
    struct = _tile_structure(e_by_batch)

    nc = bacc.Bacc("TRN2", debug=False)
    x_d = nc.dram_tensor("x_in", [T, D], BF16, kind="ExternalInput").ap()
    w_d = nc.dram_tensor("w_in", [D, 12 * 128], BF16, kind="ExternalInput").ap()
    wo_d = nc.dram_tensor("wout_in", [NH * HD, D], BF16, kind="ExternalInput").ap()
    cos_d = nc.dram_tensor("cosT_in", [128, T], BF16, kind="ExternalInput").ap()
    sin_d = nc.dram_tensor("sinS_in", [128, T], BF16, kind="ExternalInput").ap()
    e_d = nc.dram_tensor("e_in", [128, NJB * TT], F32, kind="ExternalInput").ap()
    out_d = nc.dram_tensor("out_p", [T, D], BF16, kind="ExternalOutput").ap()

    _cp = [0]

    def copy_any(out, in_):
        # PSUM-evacuation copies: only Act/DVE may read PSUM
        _cp[0] ^= 1
        if _cp[0]:
            nc.scalar.copy(out, in_)
        else:
            nc.vector.tensor_copy(out, in_)

    with tile.TileContext(nc) as tc:
        from contextlib import ExitStack

        with ExitStack() as ctx:
            consts = ctx.enter_context(tc.tile_pool(name="consts", bufs=1))
            pp = ctx.enter_context(tc.tile_pool(name="pp", bufs=1, space="PSUM"))
            wpool = ctx.enter_context(tc.tile_pool(name="wpool", bufs=1))
            kvpool = ctx.enter_context(tc.tile_pool(name="kvpool", bufs=1))
            xtp = ctx.enter_context(tc.tile_pool(name="xt", bufs=2))
            qt_pool = ctx.enter_context(tc.tile_pool(name="qt_pool", bufs=2))
            ot_pool = ctx.enter_context(tc.tile_pool(name="ot_pool", bufs=2))
            rope_pool = ctx.enter_context(tc.tile_pool(name="rope", bufs=2))
            pt_pool = ctx.enter_context(tc.tile_pool(name="pt_pool", bufs=6))
            mask_pool = ctx.enter_context(tc.tile_pool(name="mask", bufs=7))
            small = ctx.enter_context(tc.tile_pool(name="small", bufs=2))
            fin_pool = ctx.enter_context(tc.tile_pool(name="fin", bufs=3))

            # ---- constants (one-time) ----
            ones_bf = consts.tile([128, 1], BF16)
            nc.vector.memset(ones_bf, 1.0)
            cosT = consts.tile([128, T], BF16)
            sinS = consts.tile([128, T], BF16)
            e_sb = consts.tile([128, NJB * TT], F32)
            nc.sync.dma_start(e_sb, e_d)
            # integer iota compared against integer doc-ends directly
            iota_l = consts.tile([128, 512], I32)
            nc.gpsimd.iota(
                iota_l, pattern=[[1, 512]], base=0, channel_multiplier=0
            )

            pending = None  # deferred output projection (outt, tt, wout)
            for _rep in range(repeat):
                # x rows for the first group go out first so the PE can
                # start transposing ASAP; the q/k weight columns stream
                # per-kb behind them (the first projection chunk tracks the
                # stream), then v columns, then W_out (needed much later).
                xt0 = xtp.tile([128, KB, 512], BF16, tag="xt")
                nc.sync.dma_start(
                    xt0[:, 0 : KB // 2, :], x_d[0:512, 0 : D // 2],
                    transpose=True,
                )
                nc.sync.dma_start(
                    xt0[:, KB // 2 :, :], x_d[0:512, D // 2 :],
                    transpose=True,
                )
                if _rep == 0:
                    # RoPE tables land after the first x rows but before the
                    # q/k weights finish streaming
                    nc.sync.dma_start(cosT, cos_d)
                    nc.sync.dma_start(sinS, sin_d)
                if pending is not None:
                    # carried-over output projection of the previous rep's
                    # last group: runs while this rep's weights stream in,
                    # and its output DMAs beat them into the queue
                    emit_outproj(*pending)
                    pending = None
                w_bf = wpool.tile([128, KB, 12 * 128], BF16, tag="w")
                nc.sync.dma_start(
                    w_bf[:, :, 0 : 8 * 128],
                    w_d[:, 0 : 8 * 128].rearrange("(kb p) c -> p kb c", p=128),
                )
                nc.sync.dma_start(
                    w_bf[:, :, 8 * 128 : 12 * 128],
                    w_d[:, 8 * 128 : 12 * 128].rearrange(
                        "(kb p) c -> p kb c", p=128
                    ),
                )
                wout_bf = wpool.tile([128, NH, D], BF16, tag="wo", bufs=2)
                nc.sync.dma_start(
                    wout_bf, wo_d.rearrange("(a p) d -> p a d", p=128)
                )
                kT = kvpool.tile([128, NH, T], BF16, tag="kT")
                v_sb = kvpool.tile([128, NJB, NH * 128], BF16, tag="v_sb")

                def emit_outproj(outt, tt, wout_bf):
                    for lt in range(4):
                        tg = tt * 4 + lt
                        fin = fin_pool.tile([128, D], BF16, tag="fin")
                        for nt in range(4):
                            fp = pp.tile([128, 512], F32, tag="mm", bufs=2)
                            for h in range(NH):
                                nc.tensor.matmul(
                                    fp,
                                    lhsT=outt[:, h, lt * 128 : (lt + 1) * 128],
                                    rhs=wout_bf[:, h, nt * 512 : (nt + 1) * 512],
                                    start=(h == 0),
                                    stop=(h == NH - 1),
                                )
                            copy_any(fin[:, nt * 512 : (nt + 1) * 512], fp)
                        nc.sync.dma_start(
                            out_d[tg * 128 : (tg + 1) * 128, :], fin
                        )

                for tt in range(TT):
                    t0 = tt * 512
                    # -- x^T tile via XBAR DMA transpose:
                    # xt[:, kb, :] = x[t0:t0+512, kb-block].T --
                    if tt == 0:
                        xt_slab = xt0
                    else:
                        xt_slab = xtp.tile([128, KB, 512], BF16, tag="xt")
                        nc.sync.dma_start(
                            xt_slab[:, 0 : KB // 2, :],
                            x_d[t0 : t0 + 512, 0 : D // 2],
                            transpose=True,
                        )
                        nc.sync.dma_start(
                            xt_slab[:, KB // 2 :, :],
                            x_d[t0 : t0 + 512, D // 2 :],
                            transpose=True,
                        )

                    # -- doc-causal masks, shared across the 4 heads; built
                    # here so DVE/GpSimd do them while the PE projects --
                    tiles = struct[tt]
                    jblks = sorted(tiles)
                    widest_jb = max(
                        jblks, key=lambda j: tiles[j][2] - tiles[j][1]
                    )

                    def emit_mask(jblk):
                        _, mc0, mw_hi = tiles[jblk]
                        mw = mw_hi - mc0
                        m = mask_pool.tile([128, 512], BF16, tag="m")
                        # keep iff i < doc_end(j):  l < e[j] - tt*512
                        nc.vector.tensor_scalar(
                            m[:, mc0:mw_hi],
                            iota_l[:, mc0:mw_hi],
                            e_sb[:, jblk * TT + tt : jblk * TT + tt + 1],
                            None,
                            ALU.is_lt,
                        )
                        if jblk >= 4 * tt:
                            # causal half: keep iff i - j >= 0
                            nc.gpsimd.affine_select(
                                out=m[:, mc0:mw_hi],
                                in_=m[:, mc0:mw_hi],
                                compare_op=ALU.is_ge,
                                fill=0.0,
                                base=tt * 512 + mc0 - jblk * 128,
                                channel_multiplier=-1,
                                pattern=[[1, mw]],
                            )
                        return m

                    bound_jbs = [
                        j for j in jblks if tiles[j][0] == "bound"
                    ]
                    # masks are shared by the 4 heads when they fit in the
                    # ring; with pathological doc layouts (>7 bound tiles in
                    # one group) they are rebuilt per head instead
                    masks_shared = len(bound_jbs) <= 7
                    masks = {}
                    if masks_shared:
                        for jblk in bound_jbs:
                            masks[jblk] = emit_mask(jblk)

                    # -- q/k projections + RoPE (transposed layout) --
                    qT = qt_pool.tile([128, NH, 512], BF16, tag="qT")
                    for hl in range(NH):
                        for qk in range(2):  # 0 = q, 1 = k
                            chunk = qk * NH + hl
                            ps = pp.tile([128, 512], F32, tag="mm", bufs=2)
                            for kb in range(KB):
                                nc.tensor.matmul(
                                    ps,
                                    lhsT=w_bf[
                                        :, kb, chunk * 128 : (chunk + 1) * 128
                                    ],
                                    rhs=xt_slab[:, kb, :],
                                    start=(kb == 0),
                                    stop=(kb == KB - 1),
                                )
                            # RoPE fused on the PSUM result: cos-product on
                            # GpSimd, the rotate_half sin-products as two
                            # crossed-partition DVE muls (sign baked into the
                            # host-prepared sinS table), then one add.
                            raw = rope_pool.tile([128, 512], BF16, tag="raw")
                            nc.scalar.copy(raw, ps)
                            tmpc = rope_pool.tile([128, 512], BF16, tag="tmpc")
                            nc.gpsimd.tensor_mul(
                                tmpc, raw, cosT[:, t0 : t0 + 512]
                            )
                            sp = rope_pool.tile([128, 512], BF16, tag="sp")
                            nc.vector.tensor_mul(
                                sp[0:64, :], ps[64:128, :],
                                sinS[0:64, t0 : t0 + 512],
                            )
                            nc.vector.tensor_mul(
                                sp[64:128, :], ps[0:64, :],
                                sinS[64:128, t0 : t0 + 512],
                            )
                            dst = (
                                qT[:, hl, :]
                                if qk == 0
                                else kT[:, hl, t0 : t0 + 512]
                            )
                            nc.vector.tensor_add(dst, sp, tmpc)

                    # -- v projection (natural layout, 4 heads wide) --
                    for ts in range(4):
                        tb = tt * 4 + ts
                        ps = pp.tile([128, 512], F32, tag="mm", bufs=2)
                        for kb in range(KB):
                            nc.tensor.matmul(
                                ps,
                                lhsT=xt_slab[:, kb, ts * 128 : (ts + 1) * 128],
                                rhs=w_bf[:, kb, 8 * 128 : 12 * 128],
                                start=(kb == 0),
                                stop=(kb == KB - 1),
                            )
                        copy_any(v_sb[:, tb, :], ps)

                    # -- deferred output projection for the previous group:
                    # its matmuls cover the tail of that group's softmax
                    # normalize chain --
                    if pending is not None:
                        emit_outproj(*pending)
                        pending = None

                    # -- attention for this 512-row group, st two ahead.
                    # Tiles are width-trimmed to [c0, w_hi): columns below
                    # c0 are causally dead, columns past w_hi are past every
                    # document end.  The accumulation's first tile must run
                    # full width (start=True marks the whole PSUM zero
                    # region; its masked columns are zero anyway), so the
                    # widest tile goes first to minimize the trim loss. --
                    ordered = jblks
                    pairs = [(hl, j) for hl in range(NH) for j in ordered]

                    def bounds_of(jblk, first):
                        _, c0, w_hi = tiles[jblk]
                        return c0, w_hi

                    def emit_st(hl, jblk, first):
                        c0, w_hi = bounds_of(jblk, first)
                        st = pp.tile([128, 512], F32, tag="st", bufs=3)
                        nc.tensor.matmul(
                            st[:, 0 : w_hi - c0],
                            lhsT=kT[:, hl, jblk * 128 : (jblk + 1) * 128],
                            rhs=qT[:, hl, c0:w_hi],
                            start=True,
                            stop=True,
                        )
                        return st

                    nj = len(ordered)
                    outt = ot_pool.tile([128, NH, 512], BF16, tag="outt")
                    sts = [
                        emit_st(*pairs[0], True),
                        emit_st(*pairs[1], 1 % nj == 0),
                    ]
                    ones_ps = pv = None
                    for idx, (hl, jblk) in enumerate(pairs):
                        first = idx % nj == 0
                        last = idx % nj == nj - 1
                        if idx + 2 < len(pairs):
                            sts.append(
                                emit_st(*pairs[idx + 2], (idx + 2) % nj == 0)
                            )
                        st = sts[idx]
                        c0, w_hi = bounds_of(jblk, first)
                        w = w_hi - c0
                        slab = pt_pool.tile([128, 512], BF16, tag="pt")
                        nc.scalar.activation(
                            slab[:, 0:w], st[:, 0:w], AF.Exp, scale=SCALE
                        )
                        if tiles[jblk][0] == "bound":
                            m = masks[jblk] if masks_shared else emit_mask(jblk)
                            nc.vector.tensor_mul(
                                slab[:, 0:w], slab[:, 0:w], m[:, c0:w_hi]
                            )
                        if first:
                            ones_ps = pp.tile([1, 512], F32, tag="ones", bufs=1)
                            pv = pp.tile([128, 512], F32, tag="pv", bufs=2)
                        nc.tensor.matmul(
                            ones_ps[:, c0:w_hi], lhsT=ones_bf, rhs=slab[:, 0:w],
                            start=first, stop=last,
                        )
                        nc.tensor.matmul(
                            pv[:, c0:w_hi],
                            lhsT=v_sb[:, jblk, hl * 128 : (hl + 1) * 128],
                            rhs=slab[:, 0:w],
                            start=first,
                            stop=last,
                        )
                        if last:
                            rc = small.tile([1, 512], F32, tag="rc", bufs=1)
                            nc.vector.reciprocal(rc, ones_ps)
                            rb = small.tile([128, 512], F32, tag="rb")
                            nc.gpsimd.partition_broadcast(rb, rc)
                            nc.vector.tensor_mul(outt[:, hl, :], pv, rb)

                    pending = (outt, tt, wout_bf)
            emit_outproj(*pending)
    nc.compile()
    return nc


def _core_in_map(c, x, sin, cos, W_qkv, W_out, doc_ids):
    b = c // 4
    h0 = (c % 4) * 4
    wq = W_qkv[:, h0 * 128 : (h0 + 4) * 128]
    wk = W_qkv[:, D + h0 * 128 : D + (h0 + 4) * 128]
    wv = W_qkv[:, 2 * D + h0 * 128 : 2 * D + (h0 + 4) * 128]
    w_in = np.concatenate([wq, wk, wv], axis=1).astype(NPBF16)

    sinS = np.asarray(sin, np.float32).T.copy()
    sinS[0:64] *= -1.0

    e = _doc_ends(np.asarray(doc_ids[b])).astype(np.float32)
    # e_g[p, jblk*TT + g] = e[jblk*128 + p] - g*512
    e_g = (
        e.reshape(NJB, 128).T[:, :, None]
        - (np.arange(TT, dtype=np.float32) * 512.0)[None, None, :]
    ).reshape(128, NJB * TT)

    return {
        "x_in": np.ascontiguousarray(x[b]).astype(NPBF16),
        "w_in": np.ascontiguousarray(w_in),
        "wout_in": np.ascontiguousarray(
            W_out[h0 * 128 : (h0 + 4) * 128, :]
        ).astype(NPBF16),
        "cosT_in": np.ascontiguousarray(np.asarray(cos, np.float32).T).astype(
            NPBF16
        ),
        "sinS_in": np.ascontiguousarray(sinS).astype(NPBF16),
        "e_in": np.ascontiguousarray(e_g, dtype=np.float32),
    }


_last_results = None


def kernel(x, sin, cos, W_qkv, W_out, doc_ids):
    x = np.asarray(x, np.float32)
    sin = np.asarray(sin, np.float32)
    cos = np.asarray(cos, np.float32)
    W_qkv = np.asarray(W_qkv, np.float32)
    W_out = np.asarray(W_out, np.float32)
    doc_ids = np.asarray(doc_ids)

    nc = build_program(doc_ids)
    in_maps = [
        _core_in_map(c, x, sin, cos, W_qkv, W_out, doc_ids) for c in range(NCORES)
    ]
    res = run_bass_kernel_spmd(nc, in_maps, core_ids=list(range(NCORES)))
    global _last_results
    _last_results = res
    outs = [np.asarray(res.results[c]["out_p"], np.float32) for c in range(NCORES)]
    out = np.stack(
        [
            outs[0] + outs[1] + outs[2] + outs[3],
            outs[4] + outs[5] + outs[6] + outs[7],
        ]
    )
    return out.astype(np.float32)


# revision 48
# speedup vs baseline: 2.2469x; 1.0130x over previous
"""Doc-masked causal multi-head attention on TRN2, 8-core SPMD.

Sharding: core c -> batch b = c//4, heads [4*(c%4), 4*(c%4)+4).
Single pass over the core's 4 heads.  Host pre-converts x/W_qkv/W_out and
the RoPE tables to bf16 (and pre-transposes the tables), so weights and
activations DMA straight into SBUF with no on-device staging/transposes.
Each 512-row query group tt runs: x^T PE-transpose -> q/k projections with
fused RoPE -> v projection -> doc-causal attention (transposed S^T layout,
ones-matmul denominators) -> output projection against this core's 512
rows of W_out, DMAed out per 128-row block.  The host sums the 4 partials
per batch.

Doc masks are built once per (group, jblk) tile and shared by the 4 heads:
a tensor_scalar (i < doc_end) compare on DVE plus, for diagonal tiles, a
causal affine_select on GpSimd; each head then applies one bf16 mul.
Block-sparsity: (group, jblk) tiles outside every document's causal band
are skipped at trace time based on the actual doc_ids.

Matmuls run in bf16 (fp32 accumulation in PSUM); softmax math in fp32.
"""

import os
import sys

import numpy as np

for _p in ("/opt/trn_rl_repo", "/root/.axon_site/_ro/trn_rl_repo"):
    if os.path.isdir(_p) and _p not in sys.path:
        sys.path.append(_p)

import concourse.bass as bass
from concourse import bacc
import concourse.tile as tile
from concourse import mybir
from concourse.bass_utils import run_bass_kernel_spmd

B, T, D, H, HD = 2, 2048, 2048, 16, 128
NCORES = 8
NH = 4  # heads per core
TT = T // 512  # 4 query groups of 512 rows
KB = D // 128  # 16 contraction blocks
NJB = T // 128  # 16 j-blocks
SCALE = 1.0 / float(np.sqrt(HD))

F32 = mybir.dt.float32
F32R = mybir.dt.float32r
BF16 = mybir.dt.bfloat16
I32 = mybir.dt.int32
AF = mybir.ActivationFunctionType
ALU = mybir.AluOpType
NPBF16 = mybir.dt.np(BF16)


def _doc_ends(doc_row: np.ndarray) -> np.ndarray:
    """e[i] = one past the last index of the document containing row i."""
    e = np.zeros(T, np.int64)
    end = T
    for i in range(T - 1, -1, -1):
        if i < T - 1 and doc_row[i] != doc_row[i + 1]:
            end = i + 1
        e[i] = end
    return e


def _tile_structure(e_by_batch):
    """(group, jblk) -> (kind, c0, w_hi); tiles skippable for both batches
    are omitted.  group = 512 query rows, jblk = 128 key rows.  Columns
    outside [c0, w_hi) are fully masked for every batch: c0 from causality,
    w_hi from the last document end in the block."""
    struct = {}
    for g in range(TT):
        i_lo, i_hi = g * 512, g * 512 + 511
        tiles = {}
        for jblk in range(0, (g + 1) * 4):
            j_lo, j_hi = jblk * 128, jblk * 128 + 127
            valid = any(
                j_hi >= i_lo or int(e[j_hi]) > i_lo for e in e_by_batch
            )
            if not valid:
                continue
            full = all(
                j_hi <= i_lo and i_hi < int(e[j_lo]) for e in e_by_batch
            )
            c0 = max(0, j_lo - i_lo)
            me = max(
                int(max(e[j_lo : j_hi + 1])) for e in e_by_batch
            )
            w_hi = min(512, max(c0, me - i_lo))
            tiles[jblk] = ("full" if full else "bound", c0, w_hi)
        struct[g] = tiles
    return struct


def build_program(doc_ids: np.ndarray, repeat: int = 1):
    e_by_batch = [_doc_ends(np.asarray(doc_ids[b])) for b in range(B)]
    struct = _tile_structure(e_by_batch)

    nc = bacc.Bacc("TRN2", debug=False)
    x_d = nc.dram_tensor("x_in", [T, D], BF16, kind="ExternalInput").ap()
    w_d = nc.dram_tensor("w_in", [D, 12 * 128], BF16, kind="ExternalInput").ap()
    wo_d = nc.dram_tensor("wout_in", [NH * HD, D], BF16, kind="ExternalInput").ap()
    cos_d = nc.dram_tensor("cosT_in", [128, T], BF16, kind="ExternalInput").ap()
    sin_d = nc.dram_tensor("sinS_in", [128, T], BF16, kind="ExternalInput").ap()
    e_d = nc.dram_tensor("e_in", [128, NJB * TT], F32, kind="ExternalInput").ap()
    out_d = nc.dram_tensor("out_p", [T, D], BF16, kind="ExternalOutput").ap()

    _cp = [0]

    def copy_any(out, in_):
        # PSUM-evacuation copies: only Act/DVE may read PSUM
        _cp[0] ^= 1
        if _cp[0]:
            nc.scalar.copy(out, in_)
        else:
            nc.vector.tensor_copy(out, in_)

    with tile.TileContext(nc) as tc:
        from contextlib import ExitStack

        with ExitStack() as ctx:
            consts = ctx.enter_context(tc.tile_pool(name="consts", bufs=1))
            pp = ctx.enter_context(tc.tile_pool(name="pp", bufs=1, space="PSUM"))
            wpool = ctx.enter_context(tc.tile_pool(name="wpool", bufs=1))
            kvpool = ctx.enter_context(tc.tile_pool(name="kvpool", bufs=1))
            xtp = ctx.enter_context(tc.tile_pool(name="xt", bufs=2))
            qt_pool = ctx.enter_context(tc.tile_pool(name="qt_pool", bufs=2))
            ot_pool = ctx.enter_context(tc.tile_pool(name="ot_pool", bufs=2))
            rope_pool = ctx.enter_context(tc.tile_pool(name="rope", bufs=2))
            pt_pool = ctx.enter_context(tc.tile_pool(name="pt_pool", bufs=6))
            mask_pool = ctx.enter_context(tc.tile_pool(name="mask", bufs=7))
            small = ctx.enter_context(tc.tile_pool(name="small", bufs=2))
            fin_pool = ctx.enter_context(tc.tile_pool(name="fin", bufs=3))

            # ---- constants (one-time) ----
            ones_bf = consts.tile([128, 1], BF16)
            nc.vector.memset(ones_bf, 1.0)
            cosT = consts.tile([128, T], BF16)
            sinS = consts.tile([128, T], BF16)
            e_sb = consts.tile([128, NJB * TT], F32)
            nc.sync.dma_start(e_sb, e_d)
            # integer iota compared against integer doc-ends directly
            iota_l = consts.tile([128, 512], I32)
            nc.gpsimd.iota(
                iota_l, pattern=[[1, 512]], base=0, channel_multiplier=0
            )

            pending = None  # deferred output projection (outt, tt, wout)
            for _rep in range(repeat):
                # x rows for the first group go out first so the PE can
                # start transposing ASAP; the q/k weight columns stream
                # per-kb behind them (the first projection chunk tracks the
                # stream), then v columns, then W_out (needed much later).
                xt0 = xtp.tile([128, KB, 512], BF16, tag="xt")
                nc.sync.dma_start(
                    xt0[:, 0 : KB // 2, :], x_d[0:512, 0 : D // 2],
                    transpose=True,
                )
                nc.sync.dma_start(
                    xt0[:, KB // 2 :, :], x_d[0:512, D // 2 :],
                    transpose=True,
                )
                if _rep == 0:
                    # RoPE tables land after the first x rows but before the
                    # q/k weights finish streaming
                    nc.sync.dma_start(cosT, cos_d)
                    nc.sync.dma_start(sinS, sin_d)
                if pending is not None:
                    # carried-over output projection of the previous rep's
                    # last group: runs while this rep's weights stream in,
                    # and its output DMAs beat them into the queue
                    emit_outproj(*pending)
                    pending = None
                w_bf = wpool.tile([128, KB, 12 * 128], BF16, tag="w")
                nc.sync.dma_start(
                    w_bf[:, :, 0 : 8 * 128],
                    w_d[:, 0 : 8 * 128].rearrange("(kb p) c -> p kb c", p=128),
                )
                nc.sync.dma_start(
                    w_bf[:, :, 8 * 128 : 12 * 128],
                    w_d[:, 8 * 128 : 12 * 128].rearrange(
                        "(kb p) c -> p kb c", p=128
                    ),
                )
                wout_bf = wpool.tile([128, NH, D], BF16, tag="wo", bufs=2)
                nc.sync.dma_start(
                    wout_bf, wo_d.rearrange("(a p) d -> p a d", p=128)
                )
                kT = kvpool.tile([128, NH, T], BF16, tag="kT")
                v_sb = kvpool.tile([128, NJB, NH * 128], BF16, tag="v_sb")

                def emit_outproj(outt, tt, wout_bf):
                    for lt in range(4):
                        tg = tt * 4 + lt
                        fin = fin_pool.tile([128, D], BF16, tag="fin")
                        for nt in range(4):
                            fp = pp.tile([128, 512], F32, tag="mm", bufs=2)
                            for h in range(NH):
                                nc.tensor.matmul(
                                    fp,
                                    lhsT=outt[:, h, lt * 128 : (lt + 1) * 128],
                                    rhs=wout_bf[:, h, nt * 512 : (nt + 1) * 512],
                                    start=(h == 0),
                                    stop=(h == NH - 1),
                                )
                            copy_any(fin[:, nt * 512 : (nt + 1) * 512], fp)
                        nc.sync.dma_start(
                            out_d[tg * 128 : (tg + 1) * 128, :], fin
                        )

                for tt in range(TT):
                    t0 = tt * 512
                    # -- x^T tile via XBAR DMA transpose:
                    # xt[:, kb, :] = x[t0:t0+512, kb-block].T --
                    if tt == 0:
                        xt_slab = xt0
                    else:
                        xt_slab = xtp.tile([128, KB, 512], BF16, tag="xt")
                        nc.sync.dma_start(
                            xt_slab[:, 0 : KB // 2, :],
                            x_d[t0 : t0 + 512, 0 : D // 2],
                            transpose=True,
                        )
                        nc.sync.dma_start(
                            xt_slab[:, KB // 2 :, :],
                            x_d[t0 : t0 + 512, D // 2 :],
                            transpose=True,
                        )

                    # -- doc-causal masks, shared across the 4 heads; built
                    # here so DVE/GpSimd do them while the PE projects --
                    tiles = struct[tt]
                    jblks = sorted(tiles)
                    widest_jb = max(
                        jblks, key=lambda j: tiles[j][2] - tiles[j][1]
                    )

                    def emit_mask(jblk):
                        _, mc0, mw_hi = tiles[jblk]
                        mw = mw_hi - mc0
                        m = mask_pool.tile([128, 512], BF16, tag="m")
                        # keep iff i < doc_end(j):  l < e[j] - tt*512
                        nc.vector.tensor_scalar(
                            m[:, mc0:mw_hi],
                            iota_l[:, mc0:mw_hi],
                            e_sb[:, jblk * TT + tt : jblk * TT + tt + 1],
                            None,
                            ALU.is_lt,
                        )
                        if jblk >= 4 * tt:
                            # causal half: keep iff i - j >= 0
                            nc.gpsimd.affine_select(
                                out=m[:, mc0:mw_hi],
                                in_=m[:, mc0:mw_hi],
                                compare_op=ALU.is_ge,
                                fill=0.0,
                                base=tt * 512 + mc0 - jblk * 128,
                                channel_multiplier=-1,
                                pattern=[[1, mw]],
                            )
                        return m

                    bound_jbs = [
                        j for j in jblks if tiles[j][0] == "bound"
                    ]
                    # masks are shared by the 4 heads when they fit in the
                    # ring; with pathological doc layouts (>7 bound tiles in
                    # one group) they are rebuilt per head instead
                    masks_shared = len(bound_jbs) <= 7
                    masks = {}
                    if masks_shared:
                        for jblk in bound_jbs:
                            masks[jblk] = emit_mask(jblk)

                    # -- q/k projections + RoPE (transposed layout) --
                    qT = qt_pool.tile([128, NH, 512], BF16, tag="qT")
                    for hl in range(NH):
                        for qk in range(2):  # 0 = q, 1 = k
                            chunk = qk * NH + hl
                            ps = pp.tile([128, 512], F32, tag="mm", bufs=2)
                            for kb in range(KB):
                                nc.tensor.matmul(
                                    ps,
                                    lhsT=w_bf[
                                        :, kb, chunk * 128 : (chunk + 1) * 128
                                    ],
                                    rhs=xt_slab[:, kb, :],
                                    start=(kb == 0),
                                    stop=(kb == KB - 1),
                                )
                            # RoPE fused on the PSUM result: cos-product on
                            # GpSimd, the rotate_half sin-products as two
                            # crossed-partition DVE muls (sign baked into the
                            # host-prepared sinS table), then one add.
                            raw = rope_pool.tile([128, 512], BF16, tag="raw")
                            nc.scalar.copy(raw, ps)
                            tmpc = rope_pool.tile([128, 512], BF16, tag="tmpc")
                            nc.gpsimd.tensor_mul(
                                tmpc, raw, cosT[:, t0 : t0 + 512]
                            )
                            sp = rope_pool.tile([128, 512], BF16, tag="sp")
                            nc.vector.tensor_mul(
                                sp[0:64, :], ps[64:128, :],
                                sinS[0:64, t0 : t0 + 512],
                            )
                            nc.vector.tensor_mul(
                                sp[64:128, :], ps[0:64, :],
                                sinS[64:128, t0 : t0 + 512],
                            )
                            dst = (
                                qT[:, hl, :]
                                if qk == 0
                                else kT[:, hl, t0 : t0 + 512]
                            )
                            nc.vector.tensor_add(dst, sp, tmpc)

                    # -- v projection (natural layout, 4 heads wide) --
                    for ts in range(4):
                        tb = tt * 4 + ts
                        ps = pp.tile([128, 512], F32, tag="mm", bufs=2)
                        for kb in range(KB):
                            nc.tensor.matmul(
                                ps,
                                lhsT=xt_slab[:, kb, ts * 128 : (ts + 1) * 128],
                                rhs=w_bf[:, kb, 8 * 128 : 12 * 128],
                                start=(kb == 0),
                                stop=(kb == KB - 1),
                            )
                        copy_any(v_sb[:, tb, :], ps)

                    # -- deferred output projection for the previous group:
                    # its matmuls cover the tail of that group's softmax
                    # normalize chain --
                    if pending is not None:
                        emit_outproj(*pending)
                        pending = None

                    # -- attention for this 512-row group, st two ahead.
                    # Tiles are width-trimmed to [c0, w_hi): columns below
                    # c0 are causally dead, columns past w_hi are past every
                    # document end.  The accumulation's first tile must run
                    # full width (start=True marks the whole PSUM zero
                    # region; its masked columns are zero anyway), so the
                    # widest tile goes first to minimize the trim loss. --
                    ordered = jblks
                    pairs = [(hl, j) for hl in range(NH) for j in ordered]

                    def bounds_of(jblk, first):
                        _, c0, w_hi = tiles[jblk]
                        return c0, w_hi

                    def emit_st(hl, jblk, first):
                        c0, w_hi = bounds_of(jblk, first)
                        st = pp.tile([128, 512], F32, tag="st", bufs=4)
                        nc.tensor.matmul(
                            st[:, 0 : w_hi - c0],
                            lhsT=kT[:, hl, jblk * 128 : (jblk + 1) * 128],
                            rhs=qT[:, hl, c0:w_hi],
                            start=True,
                            stop=True,
                        )
                        return st

                    nj = len(ordered)
                    outt = ot_pool.tile([128, NH, 512], BF16, tag="outt")
                    sts = [
                        emit_st(*pairs[0], True),
                        emit_st(*pairs[1], 1 % nj == 0),
                        emit_st(*pairs[2], 2 % nj == 0),
                    ]
                    ones_ps = pv = None
                    for idx, (hl, jblk) in enumerate(pairs):
                        first = idx % nj == 0
                        last = idx % nj == nj - 1
                        if idx + 3 < len(pairs):
                            sts.append(
                                emit_st(*pairs[idx + 3], (idx + 3) % nj == 0)
                            )
                        st = sts[idx]
                        c0, w_hi = bounds_of(jblk, first)
                        w = w_hi - c0
                        slab = pt_pool.tile([128, 512], BF16, tag="pt")
                        nc.scalar.activation(
                            slab[:, 0:w], st[:, 0:w], AF.Exp, scale=SCALE
                        )
                        if tiles[jblk][0] == "bound":
                            m = masks[jblk] if masks_shared else emit_mask(jblk)
                            nc.vector.tensor_mul(
                                slab[:, 0:w], slab[:, 0:w], m[:, c0:w_hi]
                            )
                        if first:
                            ones_ps = pp.tile([1, 512], F32, tag="ones", bufs=1)
                            pv = pp.tile([128, 512], F32, tag="pv", bufs=1)
                        nc.tensor.matmul(
                            ones_ps[:, c0:w_hi], lhsT=ones_bf, rhs=slab[:, 0:w],
                            start=first, stop=last,
                        )
                        nc.tensor.matmul(
                            pv[:, c0:w_hi],
                            lhsT=v_sb[:, jblk, hl * 128 : (hl + 1) * 128],
                            rhs=slab[:, 0:w],
                            start=first,
                            stop=last,
                        )
                        if last:
                            rc = small.tile([1, 512], F32, tag="rc", bufs=1)
                            nc.vector.reciprocal(rc, ones_ps)
                            rb = small.tile([128, 512], F32, tag="rb")
                            nc.gpsimd.partition_broadcast(rb, rc)
                            nc.vector.tensor_mul(outt[:, hl, :], pv, rb)

                    pending = (outt, tt, wout_bf)
            emit_outproj(*pending)
    nc.compile()
    return nc


def _core_in_map(c, x, sin, cos, W_qkv, W_out, doc_ids):
    b = c // 4
    h0 = (c % 4) * 4
    wq = W_qkv[:, h0 * 128 : (h0 + 4) * 128]
    wk = W_qkv[:, D + h0 * 128 : D + (h0 + 4) * 128]
    wv = W_qkv[:, 2 * D + h0 * 128 : 2 * D + (h0 + 4) * 128]
    w_in = np.concatenate([wq, wk, wv], axis=1).astype(NPBF16)

    sinS = np.asarray(sin, np.float32).T.copy()
    sinS[0:64] *= -1.0

    e = _doc_ends(np.asarray(doc_ids[b])).astype(np.float32)
    # e_g[p, jblk*TT + g] = e[jblk*128 + p] - g*512
    e_g = (
        e.reshape(NJB, 128).T[:, :, None]
        - (np.arange(TT, dtype=np.float32) * 512.0)[None, None, :]
    ).reshape(128, NJB * TT)

    return {
        "x_in": np.ascontiguousarray(x[b]).astype(NPBF16),
        "w_in": np.ascontiguousarray(w_in),
        "wout_in": np.ascontiguousarray(
            W_out[h0 * 128 : (h0 + 4) * 128, :]
        ).astype(NPBF16),
        "cosT_in": np.ascontiguousarray(np.asarray(cos, np.float32).T).astype(
            NPBF16
        ),
        "sinS_in": np.ascontiguousarray(sinS).astype(NPBF16),
        "e_in": np.ascontiguousarray(e_g, dtype=np.float32),
    }


_last_results = None


def kernel(x, sin, cos, W_qkv, W_out, doc_ids):
    x = np.asarray(x, np.float32)
    sin = np.asarray(sin, np.float32)
    cos = np.asarray(cos, np.float32)
    W_qkv = np.asarray(W_qkv, np.float32)
    W_out = np.asarray(W_out, np.float32)
    doc_ids = np.asarray(doc_ids)

    nc = build_program(doc_ids)
    in_maps = [
        _core_in_map(c, x, sin, cos, W_qkv, W_out, doc_ids) for c in range(NCORES)
    ]
    res = run_bass_kernel_spmd(nc, in_maps, core_ids=list(range(NCORES)))
    global _last_results
    _last_results = res
    outs = [np.asarray(res.results[c]["out_p"], np.float32) for c in range(NCORES)]
    out = np.stack(
        [
            outs[0] + outs[1] + outs[2] + outs[3],
            outs[4] + outs[5] + outs[6] + outs[7],
        ]
    )
    return out.astype(np.float32)
